# revision 45
# baseline (speedup 1.0000x reference)
"""Trainium2 Bass kernel for nn_MultiHeadAttention_54580444397743 (v2).

Head-sharded tensor parallel over 8 NeuronCores: 2 heads/core, both batches.
vs v1 baseline (588 us -> 458 us): batch-interleaved emission (keeps the PE
dense and the HAM clock warm), 2-MM projections (fp32r hi/lo W x 12-bit x; no
x_lo input), bf16 score operand stacks (16-bit hi/lo K/Q), single-row max
subtraction (uniform row shift cancels in softmax), bf16 P/V and out-proj,
PV software-pipelined one step behind the score MMs (exp latency off the PE
critical path), out-projection units interleaved into the DVE-bound N phase,
nat-pass reduce inputs split between DVE (PSUM-direct) and ACT-copy->bf16,
combined-heads reciprocal staged off PSUM, static 8-bank PSUM plan, deep
x-prefetch, per-phase DMA queue spreading.

Per core, per 512-wide t-block tb (phases emitted b0/b1-interleaved):
  P: qp/kp/vp = W^T x (fp32r hi/lo W, 2 MMs each for Q,K; 1 for V);
     split Q,K into 12-bit hi/lo directly into the score operand stacks;
     V -> bf16 -> DMA-transposed into [V|1] PV stationaries
  N: natural-layout scores (hi-only) per 128-t-tile for the row max;
     PE-transpose of the max column into an f32r row of qx
  O(tb-1): out-projection of the previous block (fills the N-phase PE gap)
  T: S^T = [Khi;Klo]q2 + [Khi;-1][Qlo;m] per head; exp on ACT (bf16);
     PV accumulate with [V|1] (output + denominator); normalization via
     exp(-ln(den)) on ACT + PE ones-broadcast + DVE multiply
Partial y (this core's 128 rows of Wp) summed on the host (+ bias).
"""

import numpy as np
import ml_dtypes

import concourse.bass as bass
import concourse.mybir as mybir
from concourse import bacc, bass_utils
from concourse.tile import TileContext
from concourse.masks import make_identity

B, T, C = 2, 2048, 1024
H, D = 16, 64
NCORES = 8
HPC = H // NCORES          # heads per core = 2
TB = 512                   # t-block width (PSUM bank)
NTB = T // TB              # 4 t-blocks per batch
NTT = T // 128             # 16 t-tiles per batch
NCT = C // 128             # 8 c-tiles
F32 = mybir.dt.float32
F32R = mybir.dt.float32r
BF16 = mybir.dt.bfloat16
NEG = -1.0e9
USE_DMA_TRANSPOSE = False
USE_LNEXP_RECIP = False
USE_APPROX_RECIP = True


def _r12(x):
    m, e = np.frexp(np.asarray(x, np.float64))
    return np.ldexp(np.round(m * 4096.0) / 4096.0, e).astype(np.float32)


def build_nc():
    nc = bacc.Bacc(target_bir_lowering=False, debug=False)

    xh = nc.dram_tensor("xh", [B, C, T], F32R, kind="ExternalInput")
    wqh = nc.dram_tensor("wqh", [C, 128], F32R, kind="ExternalInput")
    wql = nc.dram_tensor("wql", [C, 128], F32R, kind="ExternalInput")
    wkh = nc.dram_tensor("wkh", [C, 128], F32R, kind="ExternalInput")
    wkl = nc.dram_tensor("wkl", [C, 128], F32R, kind="ExternalInput")
    wv = nc.dram_tensor("wv", [C, 128], F32R, kind="ExternalInput")
    wp = nc.dram_tensor("wp", [128, C], BF16, kind="ExternalInput")
    mask_nat = nc.dram_tensor("mask_nat", [128, 128], F32, kind="ExternalInput")
    mask_t = nc.dram_tensor("mask_t", [128, 128], F32, kind="ExternalInput")
    sel = nc.dram_tensor("sel", [2, 128], F32, kind="ExternalInput")
    negrow = nc.dram_tensor("negrow", [1, T], BF16, kind="ExternalInput")
    y = nc.dram_tensor("y", [B, T, C], F32, kind="ExternalOutput")

    EXP = mybir.ActivationFunctionType.Exp
    LN = mybir.ActivationFunctionType.Ln
    COPY = mybir.ActivationFunctionType.Copy

    with TileContext(nc) as tc:
        with (
            tc.tile_pool(name="sbW", bufs=1) as sbW,
            tc.tile_pool(name="sbBig", bufs=1) as sbBig,
            tc.tile_pool(name="sbV", bufs=1) as sbV,
            tc.tile_pool(name="sbX", bufs=10) as sbX,
            tc.tile_pool(name="sbS", bufs=1) as sbS,
            tc.tile_pool(name="sbS2", bufs=3) as sbS2,
            tc.tile_pool(name="sbP", bufs=4) as sbP,
            tc.tile_pool(name="sbY", bufs=3) as sbY,
            tc.tile_pool(name="sbO", bufs=2) as sbO,
            tc.tile_pool(name="ps", bufs=1, space="PSUM") as ps,
        ):
            # ---------------- constants / weights (resident) ----------------
            t_wqh = sbW.tile([128, C], F32R, name="t_wqh")
            t_wql = sbW.tile([128, C], F32R, name="t_wql")
            t_wkh = sbW.tile([128, C], F32R, name="t_wkh")
            t_wkl = sbW.tile([128, C], F32R, name="t_wkl")
            t_wv = sbW.tile([128, C], F32R, name="t_wv")
            t_wp = sbW.tile([128, C], BF16, name="t_wp")
            for j in range(NCT):
                cs = slice(128 * j, 128 * (j + 1))
                nc.gpsimd.dma_start(out=t_wqh[:, cs], in_=wqh[cs, :])
                nc.gpsimd.dma_start(out=t_wql[:, cs], in_=wql[cs, :])
                nc.gpsimd.dma_start(out=t_wkh[:, cs], in_=wkh[cs, :])
                nc.gpsimd.dma_start(out=t_wkl[:, cs], in_=wkl[cs, :])
                nc.gpsimd.dma_start(out=t_wv[:, cs], in_=wv[cs, :])
            nc.gpsimd.dma_start(out=t_wp[:, :], in_=wp[:, :])
            t_sel = sbW.tile([2, 128], F32, name="t_sel")
            nc.sync.dma_start(out=t_sel[:, :], in_=sel[:, :])
            t_mnat = sbW.tile([128, 128], F32, name="t_mnat")
            t_mt = sbW.tile([128, 128], F32, name="t_mt")
            nc.sync.dma_start(out=t_mnat[:, :], in_=mask_nat[:, :])
            nc.sync.dma_start(out=t_mt[:, :], in_=mask_t[:, :])
            t_idf = sbW.tile([128, 128], F32, name="t_idf")
            make_identity(nc, t_idf)
            t_idb = sbW.tile([128, 128], F32R, name="t_idb")
            nc.vector.tensor_copy(t_idb[:, :], t_idf[:, :])

            # ---------------- per-batch persistent tensors ----------------
            kc0, kc1, q20, q21 = {}, {}, {}, {}
            kx0, kx1 = {}, {}
            qx0, qx1 = {}, {}      # per-tb transient [65, TB] tiles, refs updated each tb
            vaug = {}
            for b in range(B):
                kc0[b] = sbBig.tile([128, T], BF16, name=f"kc0_{b}")
                kc1[b] = sbBig.tile([128, T], BF16, name=f"kc1_{b}")
                q20[b] = sbBig.tile([128, T], BF16, name=f"q20_{b}")
                q21[b] = sbBig.tile([128, T], BF16, name=f"q21_{b}")
                kx0[b] = sbBig.tile([65, T], BF16, name=f"kx0_{b}")
                kx1[b] = sbBig.tile([65, T], BF16, name=f"kx1_{b}")
                nc.sync.dma_start(out=kx0[b][64:65, :], in_=negrow[:, :])
                nc.sync.dma_start(out=kx1[b][64:65, :], in_=negrow[:, :])
                vaug[b] = [
                    sbV.tile([128, 130], BF16, name=f"vaug_{b}_{st}")
                    for st in range(NTT)
                ]

            onorm_ref = {}
            norm_ref = {}

            def emit_norm_B():
                # stage B: recip + partition-broadcast + multiply (off-PSUM)
                for b in range(B):
                    if b not in norm_ref:
                        continue
                    ost, dsb, tbn = norm_ref.pop(b)
                    rcpd = sbS2.tile([66, TB], F32, name=f"rcp_{b}_{tbn}", tag="rcpT")
                    with nc.allow_low_precision(reason="softmax recip"):
                        nc.vector.reciprocal(rcpd[64:66, :], dsb[64:66, :])
                    rb = sbS2.tile([2, TB], F32, name=f"rb_{b}_{tbn}", tag="rbT")
                    nc.gpsimd.dma_start(out=rb[0:1, :], in_=rcpd[64:65, :])
                    nc.gpsimd.dma_start(out=rb[1:2, :], in_=rcpd[65:66, :])
                    bc = ps.tile([128, TB], F32, name=f"bc_{b}_{tbn}", tag=f"s3_{b}")
                    nc.tensor.matmul(bc[:, :], t_sel[:, :], rb[:, :],
                                     start=True, stop=True)
                    onorm = sbO.tile([128, TB], BF16, name=f"on_{b}_{tbn}",
                                     tag=f"on_{b}")
                    nc.vector.tensor_mul(onorm[:, :], ost[:, :], bc[:, :])
                    onorm_ref[(b, tbn)] = onorm

            # ================= phase emitters =================
            def emit_P(tb):
                ts = slice(TB * tb, TB * (tb + 1))
                qp, kp, vp = {}, {}, {}
                for b in range(B):
                    qp[b] = ps.tile([128, TB], F32, name=f"qp_{b}_{tb}", tag=f"s0_{b}")
                    kp[b] = ps.tile([128, TB], F32, name=f"kp_{b}_{tb}", tag=f"s1_{b}")
                    vp[b] = ps.tile([128, TB], F32, name=f"vp_{b}_{tb}", tag=f"s2_{b}")
                for j in range(NCT):
                    cs = slice(128 * j, 128 * (j + 1))
                    tx = {}
                    for b in range(B):
                        tx[b] = sbX.tile([128, TB], F32R, name=f"tx_{b}_{tb}_{j}",
                                         tag=f"xh_{b}")
                        eng = nc.sync if b == 0 else nc.gpsimd
                        eng.dma_start(out=tx[b][:, :], in_=xh[b, cs, ts])
                    st_, sp_ = (j == 0), (j == NCT - 1)
                    for w_t, dst, first, last in (
                        (t_wqh, qp, st_, False),
                        (t_wql, qp, False, sp_),
                        (t_wkh, kp, st_, False),
                        (t_wkl, kp, False, sp_),
                        (t_wv, vp, st_, sp_),
                    ):
                        for b in range(B):
                            nc.tensor.matmul(dst[b][:, :], w_t[:, cs], tx[b][:, :],
                                             start=first, stop=last)
                # post-projection: hi/lo splits + operand assembly
                for b in range(B):
                    ve = nc.vector
                    stgA = sbS.tile([128, TB], BF16, name=f"sa_{b}_{tb}", tag=f"sa_{b}")
                    stgB = sbS.tile([128, TB], BF16, name=f"sb_{b}_{tb}", tag=f"sb_{b}")
                    qx0[b] = sbS2.tile([65, TB], BF16, name=f"qx0_{b}_{tb}",
                                       tag=f"qx0_{b}")
                    qx1[b] = sbS2.tile([65, TB], BF16, name=f"qx1_{b}_{tb}",
                                       tag=f"qx1_{b}")
                    ve.tensor_copy(q20[b][0:64, ts], qp[b][0:64, :])
                    ve.tensor_copy(q21[b][64:128, ts], qp[b][64:128, :])
                    ve.tensor_sub(qx0[b][0:64, :], qp[b][0:64, :],
                                  q20[b][0:64, ts])
                    ve.tensor_sub(stgA[64:128, :], qp[b][64:128, :],
                                  q21[b][64:128, ts])
                    ve.tensor_copy(kc0[b][0:64, ts], kp[b][0:64, :])
                    ve.tensor_copy(kc1[b][64:128, ts], kp[b][64:128, :])
                    ve.tensor_sub(stgA[0:64, :], kp[b][0:64, :],
                                  kc0[b][0:64, ts])
                    ve.tensor_sub(stgB[64:128, :], kp[b][64:128, :],
                                  kc1[b][64:128, ts])
                    # q-side duplications / shifts
                    nc.gpsimd.dma_start(out=q20[b][64:128, ts], in_=q20[b][0:64, ts])
                    nc.gpsimd.dma_start(out=q21[b][0:64, ts], in_=q21[b][64:128, ts])
                    nc.gpsimd.dma_start(out=qx1[b][0:64, :], in_=stgA[64:128, :])
                    # k-side shifts on sync
                    nc.sync.dma_start(out=kc0[b][64:128, ts], in_=stgA[0:64, :])
                    nc.sync.dma_start(out=kc1[b][0:64, ts], in_=stgB[64:128, :])
                    nc.sync.dma_start(out=kx0[b][0:64, ts], in_=kc0[b][0:64, ts])
                    nc.sync.dma_start(out=kx1[b][0:64, ts], in_=kc1[b][64:128, ts])
                    # V -> bf16 -> transposed into [V|1] stationaries
                    vtr = sbS.tile([128, TB], F32R if not USE_DMA_TRANSPOSE else BF16,
                                   name=f"vtr_{b}_{tb}", tag=f"vtr_{b}")
                    ve.tensor_copy(vtr[:, :], vp[b][:, :])
                    if USE_DMA_TRANSPOSE:
                        teng = nc.sync if b == 0 else nc.scalar
                        for i in range(4):
                            st = 4 * tb + i
                            va = vaug[b][st]
                            ss = slice(128 * i, 128 * (i + 1))
                            teng.dma_start_transpose(va[:, 0:64], vtr[0:64, ss])
                            teng.dma_start_transpose(va[:, 65:129], vtr[64:128, ss])
                            ve.memset(va[:, 64:65], 1.0)
                            ve.memset(va[:, 129:130], 1.0)
                    else:
                        for i in range(4):
                            st = 4 * tb + i
                            va = vaug[b][st]
                            ss = slice(128 * i, 128 * (i + 1))
                            tvp = ps.tile([128, 128], F32R, name=f"tv_{b}_{tb}_{i}",
                                          tag=f"s3_{b}")
                            nc.tensor.transpose(tvp[:, :], vtr[:, ss], t_idb[:, :])
                            ve.tensor_copy(va[:, 0:64], tvp[:, 0:64])
                            ve.tensor_copy(va[:, 65:129], tvp[:, 64:128])
                            ve.memset(va[:, 64:65], 1.0)
                            ve.memset(va[:, 129:130], 1.0)

                emit_norm_B()

            def emit_N(tb):
                ts = slice(TB * tb, TB * (tb + 1))
                ounits = []
                if tb > 0:
                    ounits = [(tb - 1, tl, b, e)
                              for tl in range(4) for b in range(B) for e in range(2)]
                m4 = {}
                for b in range(B):
                    m4[b] = ps.tile([2, TB], F32, name=f"m4_{b}_{tb}", tag=f"s2_{b}")
                for i in range(4):
                    ti = 4 * tb + i
                    tts = slice(128 * ti, 128 * (ti + 1))
                    isl = slice(128 * i, 128 * (i + 1))
                    nb = ti // 4 + 1
                    msc0, msc1, mtb = {}, {}, {}
                    for b in range(B):
                        msc0[b] = sbS2.tile([128, 4], F32, name=f"m0_{b}_{ti}",
                                            tag=f"msc0_{b}")
                        msc1[b] = sbS2.tile([128, 4], F32, name=f"m1_{b}_{ti}",
                                            tag=f"msc1_{b}")
                    for j in range(nb):
                        w = 512 if j < nb - 1 else 128 * (ti + 1) - 512 * (nb - 1)
                        ss = slice(512 * j, 512 * j + w)
                        for b in range(B):
                            aeng = nc.vector
                            np0 = ps.tile([128, TB], F32, name=f"n0_{b}_{ti}_{j}",
                                          tag=f"s0_{b}")
                            np1 = ps.tile([128, TB], F32, name=f"n1_{b}_{ti}_{j}",
                                          tag=f"s1_{b}")
                            nc.tensor.matmul(np0[:, :w], q20[b][0:64, tts],
                                             kc0[b][0:64, ss], start=True, stop=True,
                                             tile_position=(0, 0))
                            nc.tensor.matmul(np1[:, :w], q21[b][64:128, tts],
                                             kc1[b][64:128, ss], start=True, stop=True,
                                             tile_position=(64, 0))
                            if j == nb - 1:
                                dsl = slice(w - 128, w)
                                aeng.tensor_add(np0[:, dsl], np0[:, dsl], t_mnat[:, :])
                                aeng.tensor_add(np1[:, dsl], np1[:, dsl], t_mnat[:, :])
                            if (j + b) % 2 == 0:
                                nc.vector.reduce_max(msc0[b][:, j:j + 1], np0[:, :w],
                                                     axis=mybir.AxisListType.X)
                                nc.vector.reduce_max(msc1[b][:, j:j + 1], np1[:, :w],
                                                     axis=mybir.AxisListType.X)
                            else:
                                nb0 = sbS2.tile([128, TB], BF16,
                                                name=f"nb0_{b}_{ti}_{j}",
                                                tag=f"nb_{b}")
                                nb1 = sbS2.tile([128, TB], BF16,
                                                name=f"nb1_{b}_{ti}_{j}",
                                                tag=f"nb_{b}")
                                nc.scalar.activation(nb0[:, :w], np0[:, :w], COPY)
                                nc.scalar.activation(nb1[:, :w], np1[:, :w], COPY)
                                nc.vector.reduce_max(msc0[b][:, j:j + 1], nb0[:, :w],
                                                     axis=mybir.AxisListType.X)
                                nc.vector.reduce_max(msc1[b][:, j:j + 1], nb1[:, :w],
                                                     axis=mybir.AxisListType.X)
                    for b in range(B):
                        mtb[b] = sbS2.tile([128, 2], F32, name=f"mtb_{b}_{ti}",
                                           tag=f"mtb_{b}")
                        nc.vector.reduce_max(mtb[b][:, 0:1], msc0[b][:, 0:nb],
                                             axis=mybir.AxisListType.X)
                        nc.vector.reduce_max(mtb[b][:, 1:2], msc1[b][:, 0:nb],
                                             axis=mybir.AxisListType.X)
                        nc.tensor.transpose(m4[b][0:2, isl], mtb[b][:, :], t_idf[:, :])
                    for _ in range(4):
                        if ounits:
                            tbo_, tl_, b_, e_ = ounits.pop(0)
                            o_unit(tbo_, tl_, b_, e_, f"s3_{b_}")
                for b in range(B):
                    mrow = sbS.tile([2, TB], BF16, name=f"mr_{b}_{tb}", tag=f"mr_{b}")
                    nc.vector.tensor_copy(mrow[:, :], m4[b][:, :])
                    nc.gpsimd.dma_start(out=qx0[b][64:65, :], in_=mrow[0:1, :])
                    nc.gpsimd.dma_start(out=qx1[b][64:65, :], in_=mrow[1:2, :])

            def emit_T(tb):
                t0 = TB * tb
                ts = slice(t0, t0 + TB)
                last = 4 * (tb + 1) - 1
                ov0, ov1 = {}, {}
                for b in range(B):
                    ov0[b] = ps.tile([128, TB], F32, name=f"ov0_{b}_{tb}", tag=f"s2_{b}")
                    ov1[b] = ps.tile([128, TB], F32, name=f"ov1_{b}_{tb}", tag=f"s3_{b}")
                pending = {}

                def flush_pv(b):
                    if b not in pending:
                        return
                    pst, ppt0, ppt1, psl = pending.pop(b)
                    va = vaug[b][pst]
                    nc.tensor.matmul(ov0[b][0:65, psl], va[:, 0:65], ppt0[:, psl],
                                     start=(pst == 0), stop=(pst == last))
                    nc.tensor.matmul(ov1[b][0:65, psl], va[:, 65:130], ppt1[:, psl],
                                     start=(pst == 0), stop=(pst == last))

                for st in range(4 * (tb + 1)):
                    diag = st >= 4 * tb
                    coff = 128 * st - t0 if diag else 0
                    sl = slice(coff, TB)
                    tsl = slice(t0 + coff, t0 + TB)
                    sts = slice(128 * st, 128 * (st + 1))
                    for b in range(B):
                        ve = nc.vector
                        sp0 = ps.tile([128, TB], F32, name=f"p0_{b}_{tb}_{st}",
                                      tag=f"s0_{b}")
                        sp1 = ps.tile([128, TB], F32, name=f"p1_{b}_{tb}_{st}",
                                      tag=f"s1_{b}")
                        nc.tensor.matmul(sp0[:, sl], kc0[b][:, sts], q20[b][:, tsl],
                                         start=True, stop=False)
                        nc.tensor.matmul(sp0[:, sl], kx0[b][:, sts], qx0[b][:, sl],
                                         start=False, stop=True)
                        nc.tensor.matmul(sp1[:, sl], kc1[b][:, sts], q21[b][:, tsl],
                                         start=True, stop=False)
                        nc.tensor.matmul(sp1[:, sl], kx1[b][:, sts], qx1[b][:, sl],
                                         start=False, stop=True)
                        if diag:
                            dsl = slice(coff, coff + 128)
                            ve.tensor_add(sp0[:, dsl], sp0[:, dsl], t_mt[:, :])
                            ve.tensor_add(sp1[:, dsl], sp1[:, dsl], t_mt[:, :])
                        # PV of the previous step first (its exp is long done)
                        flush_pv(b)
                        pt0 = sbP.tile([128, TB], BF16, name=f"e0_{b}_{tb}_{st}",
                                       tag="pt0")
                        pt1 = sbP.tile([128, TB], BF16, name=f"e1_{b}_{tb}_{st}",
                                       tag="pt1")
                        nc.scalar.activation(pt0[:, sl], sp0[:, sl], EXP)
                        nc.scalar.activation(pt1[:, sl], sp1[:, sl], EXP)
                        pending[b] = (st, pt0, pt1, sl)
                        # HAM keep-warm: harmless PE-array activity while the
                        # next score MMs wait on the exp ring (each real MM
                        # reloads its own stationary, so this clobbers nothing)
                        nc.tensor.ldweights(t_wp[:, 0:128])
                        nc.tensor.ldweights(t_wp[:, 128:256])
                for b in range(B):
                    flush_pv(b)
                # ---- normalization stage A: drain PSUM (unnorm out + dens) ----
                for b in range(B):
                    ost = sbS.tile([128, TB], F32, name=f"os_{b}_{tb}", tag=f"ost_{b}")
                    stg1 = sbS.tile([65, TB], F32, name=f"sg_{b}_{tb}", tag="stg1")
                    dsb = sbS2.tile([66, TB], F32, name=f"dn_{b}_{tb}", tag="denT")
                    nc.scalar.activation(ost[0:64, :], ov0[b][0:64, :], COPY)
                    nc.scalar.activation(stg1[0:64, :], ov1[b][0:64, :], COPY)
                    nc.sync.dma_start(out=ost[64:128, :], in_=stg1[0:64, :])
                    nc.scalar.activation(dsb[64:65, :], ov0[b][64:65, :], COPY)
                    nc.scalar.activation(stg1[64:65, :], ov1[b][64:65, :], COPY)
                    nc.sync.dma_start(out=dsb[65:66, :], in_=stg1[64:65, :])
                    norm_ref[b] = (ost, dsb, tb)

            def o_unit(tbo, tl, b, e, slot_tag):
                tt = 4 * tbo + tl
                tts = slice(128 * tt, 128 * (tt + 1))
                osl = slice(128 * tl, 128 * (tl + 1))
                es = slice(512 * e, 512 * (e + 1))
                onorm = onorm_ref[(b, tbo)]
                yp = ps.tile([128, 512], F32, name=f"yp_{b}_{tt}_{e}", tag=slot_tag)
                nc.tensor.matmul(yp[:, :], onorm[:, osl], t_wp[:, es],
                                 start=True, stop=True)
                ysb = sbY.tile([128, 512], F32, name=f"ys_{b}_{tt}_{e}", tag="ysb")
                nc.scalar.activation(ysb[:, :], yp[:, :], COPY)
                eng = nc.sync if e == 0 else nc.gpsimd
                eng.dma_start(out=y[b, tts, es], in_=ysb[:, :])

            def emit_O(tbo):
                for tl in range(4):
                    for b in range(B):
                        for e in range(2):
                            o_unit(tbo, tl, b, e, f"s3_{b}")

            # ================= pipeline =================
            for tb in range(NTB):
                emit_P(tb)
                emit_N(tb)
                emit_T(tb)
            emit_norm_B()
            emit_O(NTB - 1)

    nc.compile()
    return nc


_NC_CACHE = {}


def _get_nc():
    if "nc" not in _NC_CACHE:
        _NC_CACHE["nc"] = build_nc()
    return _NC_CACHE["nc"]


def prepare_in_maps(ix, Wq, Wk, Wv, Wp):
    ix = np.asarray(ix, np.float32)
    Wq = np.asarray(Wq, np.float32)
    Wk = np.asarray(Wk, np.float32)
    Wv = np.asarray(Wv, np.float32)
    Wp = np.asarray(Wp, np.float32)

    xt = np.ascontiguousarray(ix.transpose(0, 2, 1))  # [B, C, T]
    xhi = _r12(xt)

    mask = np.where(np.arange(128)[:, None] >= np.arange(128)[None, :], 0.0, NEG).astype(np.float32)

    in_maps = []
    for core in range(NCORES):
        h0, h1 = HPC * core, HPC * core + 1
        wq_b = np.concatenate([Wq[h0], Wq[h1]], axis=1) * 32.0   # [C, 128], carries sqrt(C)
        wk_b = np.concatenate([Wk[h0], Wk[h1]], axis=1)
        wv_b = np.concatenate([Wv[h0], Wv[h1]], axis=1)
        wqh = _r12(wq_b)
        wql = _r12(wq_b.astype(np.float64) - wqh)
        wkh = _r12(wk_b)
        wkl = _r12(wk_b.astype(np.float64) - wkh)
        in_maps.append({
            "xh": xhi,
            "wqh": wqh, "wql": wql, "wkh": wkh, "wkl": wkl,
            "wv": _r12(wv_b),
            "wp": Wp[D * h0:D * (h1 + 1), :].astype(ml_dtypes.bfloat16),
            "mask_nat": mask, "mask_t": np.ascontiguousarray(mask.T),
            "sel": np.repeat(np.eye(2, dtype=np.float32), 64, axis=1),
            "negrow": np.full((1, T), -1.0, ml_dtypes.bfloat16),
        })
    return in_maps


def kernel(ix, Wq, Wk, Wv, Wp, bp):
    in_maps = prepare_in_maps(ix, Wq, Wk, Wv, Wp)
    bp = np.asarray(bp, np.float32)
    nc = _get_nc()
    res = bass_utils.run_bass_kernel_spmd(nc, in_maps, list(range(NCORES)))
    out = np.zeros((B, T, C), np.float64)
    for r in res.results:
        out += r["y"]
    out += bp
    return out.astype(np.float32)
